# revision 2
# baseline (speedup 1.0000x reference)
"""Multi-head attention (N=2, L=2048, D=1024, H=16) on 8 NeuronCores.

Sharding: core c -> (batch n = c // 4, head group g = c % 4, 4 heads each).
Each core computes Q/K/V projections for its 4 heads, attention, and its
slice of the output projection. Host sums the 4 partial output projections
per batch and adds bo.

v2 design (vs baseline):
- S^T matmuls are row-tiled: per head K=64 (real head_dim), head pairs run
  concurrently in PE row groups (0,0)/(64,0) instead of zero-padding Q to
  K=128.
- PV matmuls are col-tiled: M=64 per head, head pairs land at PSUM
  partitions 0:64 / 64:128 of one bank via tile_position (0,0)/(0,64).
- Softmax denominators come from 4-way col-tiled M=1 matmuls (ones lhsT)
  accumulating at PSUM partitions {0,32,64,96} of a dedicated bank that is
  memset to 1.0 once (so reciprocal of unused rows is safe), then
  reciprocal on DVE + a selector matmul broadcasts per-head 1/denom.
- Output projection contracts over a full 128-partition head pair (no
  zero padding), out DMA in fp16.

All matmul operands are fp16 (full-speed PE, fp32 PSUM accumulate).
"""
import os
import sys
import types

import numpy as np

N_BATCH = 2
L = 2048
D = 1024
H = 16
HD = 64
CORES = 8
GH = 4            # heads per core
DG = GH * HD      # 256 = projected dims per core
QB = 512          # q block
KT = L // 128     # 16 k tiles
QT = L // QB      # 4 q blocks
DC = D // 128     # 8 din chunks
SCALE = 1.0 / 32.0  # 1/sqrt(D)


def _install_ntff_hook():
    """The image's antenv stub lacks axon_hooks; shim it so trace=True works."""
    if "antenv.axon_hooks" in sys.modules:
        return
    mod = types.ModuleType("antenv.axon_hooks")
    mod._hook = None
    mod.set_axon_ntff_profile_hook = lambda h: setattr(mod, "_hook", h)
    mod.get_axon_ntff_profile_hook = lambda: mod._hook
    sys.modules["antenv.axon_hooks"] = mod
    try:
        from trn_agent_boot.trn_boot import _ntff_profile_via_ctypes
        mod._hook = _ntff_profile_via_ctypes("/opt/axon/libaxon_pjrt.so")
    except Exception:
        mod._hook = None


_install_ntff_hook()

import concourse.bacc as bacc
import concourse.mybir as mybir
import concourse.tile as tile
from concourse.bass_utils import run_bass_kernel_spmd

F32 = mybir.dt.float32
F16 = mybir.dt.float16
AF = mybir.ActivationFunctionType
MULT = mybir.AluOpType.mult

_CACHE = {}


def _build(use_bias, use_mask):
    key = (use_bias, use_mask)
    if key in _CACHE:
        return _CACHE[key]

    nc = bacc.Bacc("TRN2", debug=False, num_devices=CORES)

    xqT = nc.dram_tensor("xqT", [D, L], F16, kind="ExternalInput").ap()
    xkT = nc.dram_tensor("xkT", [D, L], F16, kind="ExternalInput").ap()
    xvT = nc.dram_tensor("xvT", [D, L], F16, kind="ExternalInput").ap()
    aq = nc.dram_tensor("aq", [128, DC * DG], F16, kind="ExternalInput").ap()
    ak = nc.dram_tensor("ak", [128, DC * DG], F16, kind="ExternalInput").ap()
    av = nc.dram_tensor("av", [128, DC * DG], F16, kind="ExternalInput").ap()
    bo = nc.dram_tensor("bo", [128, 2 * D], F16, kind="ExternalInput").ap()
    sel = nc.dram_tensor("sel", [128, 256], F16, kind="ExternalInput").ap()
    bq = nc.dram_tensor("bq", [1, DG], F16, kind="ExternalInput").ap()
    bk = nc.dram_tensor("bk", [1, DG], F16, kind="ExternalInput").ap()
    bv = nc.dram_tensor("bv", [1, DG], F16, kind="ExternalInput").ap()
    maskf = nc.dram_tensor("maskf", [128, KT], F32, kind="ExternalInput").ap()
    mask16 = nc.dram_tensor("mask16", [128, KT], F16, kind="ExternalInput").ap()
    onesd = nc.dram_tensor("onesd", [128, 512], F16, kind="ExternalInput").ap()
    outp = nc.dram_tensor("outp", [L, D], F16, kind="ExternalOutput").ap()

    with tile.TileContext(nc) as tc:
        _emit(nc, tc, dict(xqT=xqT, xkT=xkT, xvT=xvT, aq=aq, ak=ak, av=av,
                           bo=bo, sel=sel, bq=bq, bk=bk, bv=bv, maskf=maskf,
                           mask16=mask16, onesd=onesd, outp=outp),
              use_bias, use_mask)
    nc.compile()
    _CACHE[key] = nc
    return nc


def _emit(nc, tc, t, use_bias, use_mask):
    from contextlib import ExitStack
    ctx = ExitStack()
    with ctx:
        sb_w = ctx.enter_context(tc.tile_pool(name="sb_w", bufs=1))
        sb_qkv = ctx.enter_context(tc.tile_pool(name="sb_qkv", bufs=1))
        sb_pt = ctx.enter_context(tc.tile_pool(name="sb_pt", bufs=6))
        sb_n = ctx.enter_context(tc.tile_pool(name="sb_n", bufs=4))
        sb_out = ctx.enter_context(tc.tile_pool(name="sb_out", bufs=3))
        ps = ctx.enter_context(tc.tile_pool(name="ps", bufs=8, space="PSUM"))

        # ---- resident tiles ----
        ak_t = sb_w.tile([128, DC, DG], F16, tag="ak")
        aq_t = sb_w.tile([128, DC, DG], F16, tag="aq")
        av_t = sb_w.tile([128, DC, DG], F16, tag="av")
        bo_t = sb_w.tile([128, 2, D], F16, tag="bo")
        sel_t = sb_w.tile([128, 2, 128], F16, tag="sel")
        ones_t = sb_w.tile([128, 512], F16, tag="ones")
        xk_res = sb_w.tile([128, DC, L], F16, tag="xk")
        xq_res = sb_w.tile([128, DC, L], F16, tag="xq")
        xv_res = sb_w.tile([128, DC, L], F16, tag="xv")
        KT_sb = [sb_qkv.tile([128, L], F16, tag=f"kt{m}", name=f"KTm{m}")
                 for m in range(2)]
        QT_p = [sb_qkv.tile([128, L], F16, tag=f"qt{p}", name=f"QTp{p}")
                for p in range(2)]
        V1 = sb_qkv.tile([128, KT, GH, HD], F16, tag="v1")

        # denominator accumulator bank: rows {0,32,64,96} get sum_k P per
        # head; all other rows stay at the 1.0 memset so reciprocal is safe.
        dn = ps.tile([128, 512], F32, tag="dn", bufs=1, name="dn")
        nc.vector.memset(dn, 1.0)

        # ---- input DMAs: one priority-ordered queue (sync) ----
        # (weights are host-preswizzled to [128, free] partition-contiguous)
        nc.sync.dma_start(out=ak_t, in_=t["ak"].rearrange("p (c d) -> p c d", c=DC))
        for c in range(DC):
            nc.sync.dma_start(out=xk_res[:, c, :],
                              in_=t["xkT"][c * 128:(c + 1) * 128, :])
        nc.sync.dma_start(out=aq_t, in_=t["aq"].rearrange("p (c d) -> p c d", c=DC))
        for c in range(DC):  # qb0 slices of xq first
            nc.sync.dma_start(
                out=xq_res[:, c, 0:512], in_=t["xqT"][c * 128:(c + 1) * 128, 0:512])
        nc.sync.dma_start(out=av_t, in_=t["av"].rearrange("p (c d) -> p c d", c=DC))
        for c in range(DC):
            nc.sync.dma_start(out=xv_res[:, c, :],
                              in_=t["xvT"][c * 128:(c + 1) * 128, :])
        nc.sync.dma_start(out=ones_t, in_=t["onesd"])
        nc.sync.dma_start(out=sel_t, in_=t["sel"].rearrange("p (a b) -> p a b", a=2))
        if use_mask:
            mask_t = sb_w.tile([128, KT], F32, tag="mask")
            nc.sync.dma_start(out=mask_t, in_=t["maskf"])
            mask16_t = sb_w.tile([128, KT], F16, tag="mask16")
            nc.sync.dma_start(out=mask16_t, in_=t["mask16"])
        for qt in range(1, QT):
            for c in range(DC):
                nc.sync.dma_start(
                    out=xq_res[:, c, qt * 512:(qt + 1) * 512],
                    in_=t["xqT"][c * 128:(c + 1) * 128, qt * 512:(qt + 1) * 512])
        nc.sync.dma_start(out=bo_t, in_=t["bo"].rearrange("p (a d) -> p a d", a=2))
        bq_t = bk_t = bv_t = None
        if use_bias:
            bq_t = sb_w.tile([1, DG], F16, tag="bq")
            bk_t = sb_w.tile([1, DG], F16, tag="bk")
            bv_t = sb_w.tile([1, DG], F16, tag="bv")
            nc.sync.dma_start(out=bq_t, in_=t["bq"])
            nc.sync.dma_start(out=bk_t, in_=t["bk"])
            nc.sync.dma_start(out=bv_t, in_=t["bv"])

        # ACT table warmup (exp only)
        warm = sb_w.tile([1, 32], F32, tag="warm")
        nc.vector.memset(warm, 1.0)
        warm2 = sb_w.tile([1, 32], F32, tag="warm2")
        nc.scalar.activation(out=warm2, in_=warm, func=AF.Exp)

        # ---- emit helpers ----
        def emit_kproj(qt):
            # per m half: accumulate over c chunks -> KT_sb[m]
            psm = [ps.tile([128, 512], F32, tag="pv", bufs=2,
                           name=f"psk_{qt}_{_}") for _ in range(2)]
            for m in range(2):
                for c in range(DC):
                    xsl = xk_res[:, c, qt * 512:(qt + 1) * 512]
                    nc.tensor.matmul(
                        psm[m][:, 0:512], ak_t[:, c, m * 128:(m + 1) * 128], xsl,
                        start=(c == 0), stop=(c == DC - 1 and not use_bias))
                if use_bias:
                    nc.tensor.matmul(
                        psm[m][:, 0:512], bk_t[:, m * 128:(m + 1) * 128],
                        ones_t[0:1, :], start=False, stop=True)
                nc.vector.tensor_copy(
                    KT_sb[m][:, qt * 512:(qt + 1) * 512], psm[m][:, 0:512])

        def emit_qproj(qt, p):
            # packed head pair p: one M=128 matmul per c chunk
            psq = ps.tile([128, 512], F32, tag="t", bufs=1, name=f"psq_{qt}_{p}")
            for c in range(DC):
                xsl = xq_res[:, c, qt * 512:(qt + 1) * 512]
                nc.tensor.matmul(
                    psq[:, 0:512], aq_t[:, c, p * 128:(p + 1) * 128], xsl,
                    start=(c == 0), stop=(c == DC - 1 and not use_bias))
            if use_bias:
                nc.tensor.matmul(
                    psq[:, 0:512], bq_t[:, p * 128:(p + 1) * 128],
                    ones_t[0:1, :], start=False, stop=True)
            nc.vector.tensor_copy(
                QT_p[p][:, qt * 512:(qt + 1) * 512], psq[:, 0:512])

        def emit_vproj(ktg, jp):
            js = (jp * 2, jp * 2 + 1)
            psv = {j: ps.tile([128, 512], F32, tag="pv", bufs=2,
                              name=f"psv_{ktg}_{j}") for j in js}
            for j in js:
                for c in range(DC):
                    xsl = xv_res[:, c, ktg * 512:(ktg + 1) * 512]
                    nc.tensor.matmul(
                        psv[j][:, 0:DG], xsl[:, j * 128:(j + 1) * 128],
                        av_t[:, c, :],
                        start=(c == 0), stop=(c == DC - 1 and not use_bias))
                if use_bias:
                    nc.tensor.matmul(
                        psv[j][:, 0:DG], ones_t[0:1, 0:128], bv_t,
                        start=False, stop=True)
                kt = ktg * 4 + j
                srcv = psv[j][:, 0:DG].rearrange("p (h d) -> p h d", h=GH)
                if use_mask:
                    nc.vector.tensor_scalar_mul(
                        V1[:, kt, :, :], srcv, mask_t[:, kt:kt + 1])
                else:
                    nc.vector.tensor_copy(V1[:, kt, :, :], srcv)

        def emit_attn_sk(qb, sk, pso):
            qs0 = qb * QB
            pss = {}
            pts = {}
            # S^T: row-tiled head pairs, K=64 each
            for hp in range(2):
                for dk in range(2):
                    kt = sk * 2 + dk
                    for hh in range(2):
                        h = hp * 2 + hh
                        r0 = 64 * hh
                        if hh == 0 and dk == 0:
                            pass
                        if h not in pss:
                            pss[h] = ps.tile([128, 1024], F32, tag="s", bufs=2,
                                             name=f"pss_{qb}_{sk}_{h}")
                        nc.tensor.matmul(
                            pss[h][:, dk * 512:(dk + 1) * 512],
                            KT_sb[hp][r0:r0 + 64, kt * 128:(kt + 1) * 128],
                            QT_p[hp][r0:r0 + 64, qs0:qs0 + QB],
                            start=True, stop=True, tile_position=(r0, 0))
                for hh in range(2):
                    h = hp * 2 + hh
                    pt = sb_pt.tile([128, 1024], F16, tag="pt",
                                    name=f"pt_{qb}_{sk}_{h}")
                    nc.scalar.activation(out=pt, in_=pss[h], func=AF.Exp,
                                         scale=SCALE)
                    pts[h] = pt
            # PV: col-tiled head pairs (M=64 at cols 0/64) + denominator
            # (4-way col-tiled M=1 at cols {0,32,64,96})
            for dk in range(2):
                kt = sk * 2 + dk
                for p in range(2):
                    for hh in range(2):
                        h = p * 2 + hh
                        c0 = 64 * hh
                        nc.tensor.matmul(
                            pso[p][c0:c0 + 64, :], V1[:, kt, h, :],
                            pts[h][:, dk * 512:(dk + 1) * 512],
                            start=(kt == 0), stop=(kt == KT - 1),
                            tile_position=(0, c0))
                dlhs = mask16_t[:, kt:kt + 1] if use_mask else ones_t[:, 0:1]
                for h in range(GH):
                    nc.tensor.matmul(
                        dn[32 * h:32 * h + 1, :], dlhs,
                        pts[h][:, dk * 512:(dk + 1) * 512],
                        start=(kt == 0), stop=(kt == KT - 1),
                        tile_position=(0, 32 * h))

        oT_all = {}

        def emit_oT(qb):
            # copy PV accumulators to SBUF (frees psum); reciprocal of denoms
            oTs = []
            for p in range(2):
                oT = sb_n.tile([128, 512], F32, tag="oT", name=f"oT_{qb}_{p}")
                nc.vector.tensor_copy(oT, pso_all[qb][p])
                oTs.append(oT)
            rr = sb_n.tile([128, 512], F32, tag="rr", bufs=2, name=f"rr_{qb}")
            nc.vector.reciprocal_approx_fast(out=rr, in_=dn)
            rr16 = sb_n.tile([128, 512], F16, tag="rr16", bufs=2,
                             name=f"rr16_{qb}")
            nc.vector.tensor_copy(rr16, rr)
            oT_all[qb] = (oTs, rr16)

        def emit_tail(qb):
            qs0 = qb * QB
            oTs, rr16 = oT_all[qb]
            oNs = []
            for p in range(2):
                bc = ps.tile([128, 512], F32, tag="t", bufs=1,
                             name=f"bc_{qb}_{p}")
                nc.tensor.matmul(bc[:, 0:512], sel_t[:, p, :], rr16,
                                 start=True, stop=True)
                oN = sb_n.tile([128, 512], F16, tag="oN", name=f"oN_{qb}_{p}")
                nc.vector.tensor_tensor(oN, oTs[p], bc, op=MULT)
                oNs.append(oN)
            for mq in range(4):
                ot = sb_out.tile([128, D], F16, tag="ot", name=f"ot_{qb}_{mq}")
                for nb in range(2):
                    psout = ps.tile([128, 512], F32, tag="t", bufs=1,
                                    name=f"psout_{qb}_{mq}_{nb}")
                    for p in range(2):
                        nc.tensor.matmul(
                            psout[:, 0:512],
                            oNs[p][:, mq * 128:(mq + 1) * 128],
                            bo_t[:, p, nb * 512:(nb + 1) * 512],
                            start=(p == 0), stop=(p == 1))
                    nc.vector.tensor_copy(ot[:, nb * 512:(nb + 1) * 512],
                                          psout[:, 0:512])
                q0 = qs0 + mq * 128
                nc.gpsimd.dma_start(out=t["outp"][q0:q0 + 128, :], in_=ot)

        # ---- schedule ----
        pso_all = {}
        emit_qproj(0, 0)
        emit_qproj(0, 1)
        for g in range(4):
            emit_kproj(g)
        for g in range(4):
            emit_vproj(g, 0)
            emit_vproj(g, 1)
        pso_all[0] = [ps.tile([128, 512], F32, tag="pv", bufs=2,
                              name=f"pso_0_{_}") for _ in range(2)]
        for sk in range(KT // 2):
            emit_attn_sk(0, sk, pso_all[0])
        emit_oT(0)
        for qb in range(1, QT):
            emit_qproj(qb, 0)
            emit_qproj(qb, 1)
            pso_all[qb] = [ps.tile([128, 512], F32, tag="pv", bufs=2,
                                   name=f"pso_{qb}_{_}") for _ in range(2)]
            for sk in range(KT // 2):
                emit_attn_sk(qb, sk, pso_all[qb])
            emit_oT(qb)
            emit_tail(qb - 1)
        emit_tail(QT - 1)


def _swizzle_a(aT):
    """[D, DG] -> [128, DC*DG]: partition p holds chunks c at (c, :)."""
    return np.ascontiguousarray(
        aT.reshape(DC, 128, DG).transpose(1, 0, 2).reshape(128, DC * DG))


def _make_sel():
    """[128, 256] selector: col p*128+m picks denom row 32*(2p + m//64)."""
    sel = np.zeros((128, 2, 128), dtype=np.float16)
    for p in range(2):
        for m in range(128):
            sel[32 * (2 * p + m // 64), p, m] = 1.0
    return np.ascontiguousarray(sel.reshape(128, 256))


_SEL = _make_sel()
_ONES = np.ones((128, 512), dtype=np.float16)


def _prep_inputs(values, key, query, mask, Wv, Wk, Wq, Wo, bv, bk, bq):
    """Build the 8 per-core input maps (host-side shard + layout)."""
    xT = {}
    for n in range(N_BATCH):
        xT[("q", n)] = np.ascontiguousarray(query[n].T.astype(np.float16))
        xT[("k", n)] = np.ascontiguousarray(key[n].T.astype(np.float16))
        xT[("v", n)] = np.ascontiguousarray(values[n].T.astype(np.float16))
    in_maps = []
    for c in range(CORES):
        n, g = divmod(c, CORES // N_BATCH)
        rows = slice(g * DG, (g + 1) * DG)
        mrow = np.ascontiguousarray(
            mask[n, 0, 0, :].astype(np.float32).reshape(KT, 128).T)
        boT = np.ascontiguousarray(
            Wo[:, rows].T.astype(np.float16).reshape(2, 128, D)
            .transpose(1, 0, 2).reshape(128, 2 * D))
        in_maps.append({
            "xqT": xT[("q", n)],
            "xkT": xT[("k", n)],
            "xvT": xT[("v", n)],
            "aq": _swizzle_a(Wq[rows, :].T.astype(np.float16)),
            "ak": _swizzle_a(Wk[rows, :].T.astype(np.float16)),
            "av": _swizzle_a(Wv[rows, :].T.astype(np.float16)),
            "bo": boT,
            "sel": _SEL,
            "bq": np.ascontiguousarray(bq[None, rows].astype(np.float16)),
            "bk": np.ascontiguousarray(bk[None, rows].astype(np.float16)),
            "bv": np.ascontiguousarray(bv[None, rows].astype(np.float16)),
            "maskf": mrow,
            "mask16": np.ascontiguousarray(mrow.astype(np.float16)),
            "onesd": _ONES,
        })
    return in_maps


LAST_EXEC_NS = None


def kernel(values, key, query, mask, Wv, bv, Wk, bk, Wq, bq, Wo, bo,
           trace=False):
    global LAST_EXEC_NS
    values = np.asarray(values, dtype=np.float32)
    key = np.asarray(key, dtype=np.float32)
    query = np.asarray(query, dtype=np.float32)
    mask = np.asarray(mask)
    Wq, Wk, Wv, Wo = (np.asarray(Wq, np.float32), np.asarray(Wk, np.float32),
                      np.asarray(Wv, np.float32), np.asarray(Wo, np.float32))
    bq, bk, bv, bo = (np.asarray(bq, np.float32), np.asarray(bk, np.float32),
                      np.asarray(bv, np.float32), np.asarray(bo, np.float32))

    use_bias = bool(np.any(bq) or np.any(bk) or np.any(bv))
    use_mask = not bool(np.all(np.asarray(mask) == 1))

    nc = _build(use_bias, use_mask)
    in_maps = _prep_inputs(values, key, query, mask, Wv, Wk, Wq, Wo,
                           bv, bk, bq)
    res = run_bass_kernel_spmd(nc, in_maps, core_ids=list(range(CORES)),
                               trace=trace)
    LAST_EXEC_NS = res.exec_time_ns

    out = np.zeros((N_BATCH, L, D), dtype=np.float32)
    for c in range(CORES):
        n = c // (CORES // N_BATCH)
        out[n] += res.results[c]["outp"].astype(np.float32)
    out += bo[None, None, :]
    return out
